# revision 24
# baseline (speedup 1.0000x reference)
# Self-contained Trainium2 Bass kernel for nn_MultiInputLSTMCell.
#
# Reference computation (all fp32):
#   pre   = h0 @ W_hh + bias + input_ @ W_ih          # (1, 3H)
#   i, o  = sigmoid(pre[:, :H]), sigmoid(pre[:, H:2H])
#   g     = tanh(pre[:, 2H:])
#   awi   = input_ @ aW_ih + a_bias                   # (1, H)
#   awh   = c_input @ aW_hh                           # (C, H)
#   alpha = sigmoid(awi + awh)                        # (C, H)
#   w     = exp([i; alpha]); w /= w.sum(0)            # (C+1, H)
#   c1    = (([g; c_input]) * w).sum(0)               # (1, H)
#   h1    = o * tanh(c1)
#
# Strategy: tensor-parallel over the hidden (output-column) dim across 8
# cores (HS = 256 columns each); everything after the matmuls is local to a
# shard, so no collectives.
#
# Key design points (from perfetto/NTFF trace analysis; baseline bf16
# kernel = 43.7us, this kernel ~29us):
#  * Gate weights host-quantized to fp8 E3M4 (4 mantissa bits), x64
#    pre-scale (x128 for the g block so one tanh(x/(2*64)) serves
#    sigmoid(i,o) and tanh(g) alike).  E3M4 streams the PE at the full
#    bf16 rate (1 col/cycle) and halves HBM bytes vs bf16; E4M3 for the
#    gates fails the 2e-2 gate (measures 2.4e-2).  End-to-end err ~1.2e-2.
#  * The alpha matmuls (x@aW_ih, c@aW_hh) run in DoubleRow (double-pumped)
#    fp8 E4M3 at 2 k-chunks per pass — the alpha path is ~3x less error-
#    sensitive than the gates, so E4M3 is safe there (operand scales:
#    x*4 @ 16*aW_ih and 2*c @ 32*aW_hh keep products at x64).
#  * All sigmoids via 0.5+0.5*tanh(x/2): tanh and exp share one ACT table
#    set, so no ~2.7us mid-kernel table reloads.
#  * PE p-states: 0.65 -> 1.2 -> 2.4 GHz with ~3us of GAPLESS activity
#    needed to reach max; any stall during the ramp resets it (a ~300ns
#    just-in-time chunk wait cost an early version 11us by pinning the
#    whole stream at 1.2 GHz).  Once at 2.4 GHz, ordinary gaps are fine
#    (clock only drops after ~3.4us idle).  19 dummy matmuls on memset
#    data bridge engine-init (~7.5us) to the first chunk semaphore
#    (~11us) with zero gaps.
#  * DMA reality on this part: each dma_start costs ~0.65us of SP
#    sequencer time, completion semaphores trail the data by ~1.3us
#    (+~3us extra for the very first transfers, absorbed here by a 64KB
#    throwaway warm-up read), and the stream sustains ~330-375 GB/s only
#    for >=0.5-1MB transfers.  So: one warm-up read, then chunk sizes
#    that grow from 0.26MB (fast first semaphore) to ~1MB, issued in
#    exact PE consumption order on the sync ring only — merely touching
#    the scalar HWDGE ring adds ~2.2us of runtime queue setup.
#  * PE order ig -> alpha -> o: the [i;alpha] exp-normalize closes as a
#    single K=65 ones-matmul, c1 and tanh(c1) are computed while the PE
#    streams the o gate, and the post-PE tail is only
#    tanh(o)->affine->mul->store.  Bias rows join as rank-1 matmuls at
#    the accumulation-group edges (no extra serial ops).
#  * Fixed costs measured on this setup: ~7us framework preamble + ~3us
#    final-DMA+teardown; a 1-DMA kernel measures 13.5us end to end.

import numpy as np

import concourse.bass as bass
import concourse.tile as tile
from concourse import bacc, mybir
from concourse.bass_utils import run_bass_kernel_spmd

NCORES = 8
H = 2048          # hidden size
IN = 2048         # input size
C = 64            # number of skip-word cell states
HS = H // NCORES  # hidden shard per core = 256
KG = IN + H       # gates contraction dim = 4096
KO_G = KG // 128  # 32 k-chunks for gates
KO_A = IN // 128  # 16 k-chunks per alpha matmul
PAIRS = KO_A // 2  # 8 DoubleRow passes per alpha matmul
SCALE = 64.0      # fp8 weight pre-scale
F32 = mybir.dt.float32
F32R = mybir.dt.float32r
BF16 = mybir.dt.bfloat16
FP8 = mybir.dt.float8e3
FP8E4 = mybir.dt.float8e4

_nc_cache = None


def _build_nc():
    """Build the single-core Bass program (same program runs on all 8 cores)."""
    nc = bacc.Bacc(
        "TRN2",
        target_bir_lowering=False,
        debug=False,
        enable_asserts=False,
        name="multi_input_lstm_cell",
    )

    # DRAM I/O (per-core shards; shapes identical on every core).  Weights
    # host-pre-tiled to [ki=128, ko, n]: chunk DMAs read one contiguous
    # segment per partition.
    # wig columns: [i-gate shard (256) | 2*g-gate shard (256)], e3m4 * 64
    wig = nc.dram_tensor("wig", [128, KO_G, 2 * HS], FP8, kind="ExternalInput").ap()
    wo = nc.dram_tensor("wo", [128, KO_G, HS], FP8, kind="ExternalInput").ap()
    # wa8: DoubleRow-interleaved alpha weights, pairs 0..7 = 16*aW_ih shard,
    # pairs 8..15 = 32*aW_hh shard (e4m3)
    wa8 = nc.dram_tensor("wa8", [128, 2 * PAIRS, 2, HS], FP8E4,
                         kind="ExternalInput").ap()
    ct8 = nc.dram_tensor("ct8", [128, PAIRS, 2, C], FP8E4,
                         kind="ExternalInput").ap()
    # xa: e4m3 x4 copy of input_ in DoubleRow pairs for the wi matmul.
    # Innermost dim padded to 16: DoubleRow LDWEIGHTS needs the k-pair
    # stride to be a multiple of 16 bytes.
    xa = nc.dram_tensor("xa", [128, PAIRS, 2, 16], FP8E4,
                        kind="ExternalInput").ap()
    # bab = [64*b_i | 128*b_g | 64*b_o | 64*ab]
    bab = nc.dram_tensor("bab", [1, 4 * HS], F32R, kind="ExternalInput").ap()
    cs = nc.dram_tensor("cs", [C, HS], F32R, kind="ExternalInput").ap()
    xt = nc.dram_tensor("xt", [128, KO_G], BF16, kind="ExternalInput").ap()
    # hc[0, 0:256] = c1 shard, hc[0, 256:512] = h1 shard
    hc = nc.dram_tensor("hc", [1, 2 * HS], F32, kind="ExternalOutput").ap()

    with tile.TileContext(nc) as tc:
        _emit(tc, wig, wo, wa8, ct8, xa, bab, cs, xt, hc)

    nc.compile()
    return nc


def _emit(tc, wig, wo, wa8, ct8, xa, bab, cs, xt, hc):
    from contextlib import ExitStack

    nc = tc.nc
    TANH = mybir.ActivationFunctionType.Tanh
    EXP = mybir.ActivationFunctionType.Exp
    MUL = mybir.AluOpType.mult
    ADD = mybir.AluOpType.add
    DR = mybir.MatmulPerfMode.DoubleRow
    INV2S = 1.0 / (2.0 * SCALE)

    with ExitStack() as ctx:
        singles = ctx.enter_context(tc.tile_pool(name="singles", bufs=1))
        psum = ctx.enter_context(tc.tile_pool(name="psum", bufs=1, space="PSUM"))

        # ---- memset-sourced tiles (no DMA dependency) --------------------
        warm_t = singles.tile([128, HS], BF16, tag="warm")
        nc.vector.memset(warm_t[:], 0.0)
        ones_b_f = singles.tile([1, C], F32, tag="ones_b")
        nc.vector.memset(ones_b_f[:], 1.0)
        ones_b = ones_b_f[:].bitcast(F32R)
        ones_r_f = singles.tile([C + 1, 1], F32, tag="ones_r")
        nc.vector.memset(ones_r_f[:], 1.0)
        ones_r = ones_r_f[:].bitcast(F32R)
        wmt = singles.tile([1, 1], F32, tag="wmt")
        nc.vector.memset(wmt[:], 0.0)
        nc.scalar.activation(out=wmt[:], in_=wmt[:], func=EXP)  # table prewarm
        half1 = singles.tile([1, 1], F32, tag="half1")
        nc.vector.memset(half1[:], 0.5)
        half64 = singles.tile([C, 1], F32, tag="half64")
        nc.vector.memset(half64[:], 0.5)

        # ---- weight/side tiles (whole tensors; DMAs write disjoint slices,
        # so no pool recycling and no backpressure on the issue stream) ----
        xt_t = singles.tile([128, KO_G], BF16, tag="xt")
        wig_t = singles.tile([128, KO_G, 2 * HS], FP8, tag="wig")
        wa8_t = singles.tile([128, 2 * PAIRS, 2, HS], FP8E4, tag="wa8")
        ct8_t = singles.tile([128, PAIRS, 2, C], FP8E4, tag="ct8")
        xa_t = singles.tile([128, PAIRS, 2, 16], FP8E4, tag="xa")
        cs_t = singles.tile([C, HS], F32R, tag="cs")
        bab_t = singles.tile([1, 4 * HS], F32R, tag="bab")
        wo_t = singles.tile([128, KO_G, HS], FP8, tag="wo")

        # sync-ring stream, in PE consumption order.  (The DMA path has
        # its own ~3us clock ramp after engine init and first-transfer
        # completion semaphores never fire before ~12us no matter how small
        # the transfer, so the schedule just rides it: small first chunks,
        # growing sizes, and the PE's dummy block sized to meet the first
        # semaphore.)
        nc.sync.dma_start(out=xt_t[:], in_=xt)
        for a, b in [(0, 4), (4, 12), (12, 22), (22, 32)]:
            nc.sync.dma_start(out=wig_t[:, a:b, :], in_=wig[:, a:b, :])
        nc.sync.dma_start(out=xa_t[:], in_=xa)
        nc.sync.dma_start(out=wa8_t[:, 0:PAIRS], in_=wa8[:, 0:PAIRS])
        nc.sync.dma_start(out=ct8_t[:], in_=ct8)
        nc.sync.dma_start(out=wa8_t[:, PAIRS : 2 * PAIRS],
                          in_=wa8[:, PAIRS : 2 * PAIRS])
        nc.sync.dma_start(out=cs_t[:], in_=cs)
        nc.sync.dma_start(out=bab_t[:], in_=bab)
        nc.sync.dma_start(out=wo_t[:, 0:24, :], in_=wo[:, 0:24, :])
        nc.sync.dma_start(out=wo_t[:, 24:32, :], in_=wo[:, 24:32, :])

        # ---- PSUM tiles --------------------------------------------------
        pg_ig = psum.tile([1, 2 * HS], F32, tag="pg_ig")  # [pre_i | 2*pre_g]
        pg_o = psum.tile([1, HS], F32, tag="pg_o")        # pre_o
        pwi = psum.tile([1, HS], F32, tag="pwi")          # alpha_wi row
        pal = psum.tile([C, HS], F32, tag="pal")          # alpha pre-activation
        ps = psum.tile([1, 2 * HS], F32, tag="ps")        # [sum_ew | sum_mg]
        pdum = psum.tile([1, HS], F32, tag="pdum")        # warm scratch

        # ---- PE ramp dummies (gap-free by construction; see header) ------
        for _ in range(24):
            nc.tensor.matmul(pdum[:], lhsT=warm_t[:, 0:1], rhs=warm_t[:],
                             start=True, stop=True)

        # ---- [i|g] gates stream (e3m4, single-rate) ----------------------
        for kk in range(KO_G):
            nc.tensor.matmul(
                pg_ig[:],
                lhsT=xt_t[:, kk : kk + 1],
                rhs=wig_t[:, kk, :],
                start=(kk == 0),
                stop=False,
            )
        # bias joins as a K=1 rank-1 matmul closing the group (keeps the
        # first gates matmul free of the bias-DMA dependency)
        nc.tensor.matmul(pg_ig[:], lhsT=ones_b[0:1, 0:1], rhs=bab_t[:, 0:512],
                         start=False, stop=True)

        # ---- ig tail: row 64 of the normalize operands -------------------
        # tio = tanh(pre/(2S)) = [t_i | tanh(pre_g)]
        tio = singles.tile([1, 2 * HS], F32, tag="tio")
        nc.scalar.activation(out=tio[:], in_=pg_ig[:], func=TANH, scale=INV2S)
        ew_t = singles.tile([C + 1, HS], F32R, tag="ew")
        mg_t = singles.tile([C + 1, HS], F32R, tag="mg")
        # ew[64] = exp(sig_i) = exp(0.5*t_i + 0.5); mg[64] = g * ew[64]
        # (two steps: DVE tensor_tensor needs equal SBUF base partitions)
        nc.scalar.activation(out=ew_t[C : C + 1, :], in_=tio[:, 0:HS], func=EXP,
                             scale=0.5, bias=half1[:])
        nc.vector.tensor_scalar(out=mg_t[C : C + 1, :], in0=tio[:, HS : 2 * HS],
                                scalar1=1.0, scalar2=None, op0=MUL)
        nc.vector.tensor_tensor(out=mg_t[C : C + 1, :], in0=mg_t[C : C + 1, :],
                                in1=ew_t[C : C + 1, :], op=MUL)

        # ---- alpha matmuls: DoubleRow fp8, 2 k-chunks per pass -----------
        for p in range(PAIRS):
            nc.tensor.matmul(
                pwi[:],
                lhsT=xa_t[:, p, :, 0:1],
                rhs=wa8_t[:, p, :, :],
                start=(p == 0),
                stop=(p == PAIRS - 1),
                perf_mode=DR,
            )
        for p in range(PAIRS):
            nc.tensor.matmul(
                pal[:],
                lhsT=ct8_t[:, p, :, :],
                rhs=wa8_t[:, PAIRS + p, :, :],
                start=(p == 0),
                stop=False,
                perf_mode=DR,
            )
        # wi row (+ alpha_bias) to SBUF, broadcast-added into pal via a K=1
        # ones matmul (closes the pal group).
        wi_t = singles.tile([1, HS], F32R, tag="wi")
        nc.vector.tensor_tensor(out=wi_t[:], in0=pwi[:], in1=bab_t[:, 768:1024],
                                op=ADD)
        nc.tensor.matmul(pal[:], lhsT=ones_b[0:1, 0:C], rhs=wi_t[:],
                         start=False, stop=True)

        # ---- alpha tail (ACT/DVE; overlaps the o stream) -----------------
        tal = singles.tile([C, HS], F32, tag="tal")
        nc.scalar.activation(out=tal[:], in_=pal[:], func=TANH, scale=INV2S)
        nc.scalar.activation(out=ew_t[0:C, :], in_=tal[:], func=EXP,
                             scale=0.5, bias=half64[:])
        nc.vector.tensor_tensor(out=mg_t[0:C, :], in0=cs_t[:], in1=ew_t[0:C, :],
                                op=MUL)

        # ---- o-gate stream with the K=65 reductions interleaved ----------
        nc.tensor.matmul(pg_o[:], lhsT=ones_b[0:1, 0:1], rhs=bab_t[:, 512:768],
                         start=True, stop=False)
        for kk in range(KO_G):
            nc.tensor.matmul(
                pg_o[:],
                lhsT=xt_t[:, kk : kk + 1],
                rhs=wo_t[:, kk, :],
                start=False,
                stop=(kk == KO_G - 1),
            )
            if kk == 21:
                nc.tensor.matmul(ps[:, 0:HS], lhsT=ones_r[:], rhs=ew_t[:],
                                 start=True, stop=True)
                nc.tensor.matmul(ps[:, HS : 2 * HS], lhsT=ones_r[:], rhs=mg_t[:],
                                 start=True, stop=True)

        # ---- c1 = ps1 / ps0 ; overlaps the o-stream tail -----------------
        r_t = singles.tile([1, HS], F32, tag="r")
        nc.vector.reciprocal_approx_fast(out=r_t[:], in_=ps[:, 0:HS])
        hc_t = singles.tile([1, 2 * HS], F32, tag="hc")
        nc.vector.tensor_tensor(out=hc_t[:, 0:HS], in0=ps[:, HS : 2 * HS],
                                in1=r_t[:], op=MUL)
        nc.sync.dma_start(out=hc[:, 0:HS], in_=hc_t[:, 0:HS])
        t4 = singles.tile([1, HS], F32, tag="t4")
        nc.scalar.activation(out=t4[:], in_=hc_t[:, 0:HS], func=TANH)

        # ---- o tail: h1 = (0.5 + 0.5*tanh(pre_o/(2S))) * tanh(c1) --------
        to_t = singles.tile([1, HS], F32, tag="to")
        nc.scalar.activation(out=to_t[:], in_=pg_o[:], func=TANH, scale=INV2S)
        nc.vector.tensor_scalar(out=to_t[:], in0=to_t[:], scalar1=0.5,
                                scalar2=0.5, op0=MUL, op1=ADD)
        nc.vector.tensor_tensor(out=hc_t[:, HS : 2 * HS], in0=to_t[:],
                                in1=t4[:], op=MUL)
        nc.sync.dma_start(out=hc[:, HS : 2 * HS], in_=hc_t[:, HS : 2 * HS])


def _shard_inputs(input_, c_input, h0, c0, weight_ih, weight_hh,
                  alpha_weight_ih, alpha_weight_hh, bias, alpha_bias):
    """Host-side scatter: column-shard the weights over the hidden dim."""
    import ml_dtypes
    f32 = np.float32
    bf16 = ml_dtypes.bfloat16
    e3m4 = ml_dtypes.float8_e3m4
    e4m3 = ml_dtypes.float8_e4m3

    x_comb = np.concatenate([h0[0], input_[0]]).astype(f32)          # (4096,)
    xt = np.ascontiguousarray(x_comb.reshape(KO_G, 128).T).astype(bf16)
    # xa: input_ chunk-pairs for the DoubleRow wi matmul, e4m3 at x4
    xa = np.zeros((128, PAIRS, 2, 16), e4m3)
    xa[:, :, :, 0] = np.ascontiguousarray(
        (input_[0].astype(f32) * f32(4.0)).reshape(PAIRS, 2, 128)
        .transpose(2, 0, 1)).astype(e4m3)

    def q8(x, sc):
        return np.clip(np.asarray(x, f32) * f32(sc), -15.5, 15.5).astype(e3m4)

    # ct8: c_input^T chunk-pairs, e4m3 at x2: [ki=128, pair=8, 2, C]
    ct8 = np.ascontiguousarray(
        (c_input.T.astype(f32) * f32(2.0)).reshape(PAIRS, 2, 128, C)
        .transpose(2, 0, 1, 3)).astype(e4m3)

    # gates weights: stack [W_hh; W_ih]; i/o at x64, g at x128 (e3m4).
    wg_full = np.concatenate([weight_hh, weight_ih], axis=0).astype(f32)
    wq_i = q8(wg_full[:, 0:H], SCALE)
    wq_o = q8(wg_full[:, H : 2 * H], SCALE)
    wq_g = q8(wg_full[:, 2 * H : 3 * H], 2 * SCALE)
    del wg_full

    def tile_k(w):  # [4096, n] -> [128, 32, n]
        n = w.shape[1]
        return np.ascontiguousarray(w.reshape(KO_G, 128, n).transpose(1, 0, 2))

    # alpha weights for DoubleRow (e4m3): ih at x16 (against x4 inputs),
    # hh at x32 (against x2 c_input) -> products uniformly x64.
    wa_q = np.concatenate(
        [(alpha_weight_ih.astype(f32) * f32(SCALE / 4)).astype(e4m3),
         (alpha_weight_hh.astype(f32) * f32(SCALE / 2)).astype(e4m3)], axis=0)

    bias = np.asarray(bias, f32)
    alpha_bias = np.asarray(alpha_bias, f32)
    c_input = np.asarray(c_input, f32)

    in_maps = []
    for k in range(NCORES):
        cols = np.s_[k * HS : (k + 1) * HS]
        wig = tile_k(np.concatenate([wq_i[:, cols], wq_g[:, cols]], axis=1))
        wo = tile_k(wq_o[:, cols])
        # [4096, 256] -> [128, 16 pairs, 2, 256]
        wa8 = np.ascontiguousarray(
            wa_q[:, cols].reshape(2 * PAIRS, 2, 128, HS).transpose(2, 0, 1, 3))
        bab = np.concatenate(
            [bias[0 * H + k * HS : 0 * H + (k + 1) * HS] * f32(SCALE),
             bias[2 * H + k * HS : 2 * H + (k + 1) * HS] * f32(2 * SCALE),
             bias[1 * H + k * HS : 1 * H + (k + 1) * HS] * f32(SCALE),
             alpha_bias[cols] * f32(SCALE)])[None, :].astype(f32)
        in_maps.append({
            "wig": wig,
            "wo": wo,
            "wa8": wa8,
            "ct8": ct8,
            "xa": xa,
            "bab": bab,
            "cs": np.ascontiguousarray(c_input[:, cols]),
            "xt": xt,
        })
    return in_maps


def _run(inputs, trace=False):
    global _nc_cache
    if _nc_cache is None:
        _nc_cache = _build_nc()
    nc = _nc_cache
    in_maps = _shard_inputs(**inputs)
    res = run_bass_kernel_spmd(nc, in_maps, core_ids=list(range(NCORES)), trace=trace)
    h1 = np.concatenate(
        [res.results[k]["hc"][:, HS : 2 * HS] for k in range(NCORES)], axis=1)
    c1 = np.concatenate(
        [res.results[k]["hc"][:, 0:HS] for k in range(NCORES)], axis=1)
    return (h1.astype(np.float32), c1.astype(np.float32)), res


def kernel(input_, c_input, h0, c0, weight_ih, weight_hh,
           alpha_weight_ih, alpha_weight_hh, bias, alpha_bias):
    inputs = dict(
        input_=np.asarray(input_, np.float32),
        c_input=np.asarray(c_input, np.float32),
        h0=np.asarray(h0, np.float32),
        c0=np.asarray(c0, np.float32),
        weight_ih=np.asarray(weight_ih, np.float32),
        weight_hh=np.asarray(weight_hh, np.float32),
        alpha_weight_ih=np.asarray(alpha_weight_ih, np.float32),
        alpha_weight_hh=np.asarray(alpha_weight_hh, np.float32),
        bias=np.asarray(bias, np.float32),
        alpha_bias=np.asarray(alpha_bias, np.float32),
    )
    out, _ = _run(inputs)
    return out


# revision 26
# speedup vs baseline: 1.0349x; 1.0349x over previous
# Self-contained Trainium2 Bass kernel for nn_MultiInputLSTMCell.
#
# Reference computation (all fp32):
#   pre   = h0 @ W_hh + bias + input_ @ W_ih          # (1, 3H)
#   i, o  = sigmoid(pre[:, :H]), sigmoid(pre[:, H:2H])
#   g     = tanh(pre[:, 2H:])
#   awi   = input_ @ aW_ih + a_bias                   # (1, H)
#   awh   = c_input @ aW_hh                           # (C, H)
#   alpha = sigmoid(awi + awh)                        # (C, H)
#   w     = exp([i; alpha]); w /= w.sum(0)            # (C+1, H)
#   c1    = (([g; c_input]) * w).sum(0)               # (1, H)
#   h1    = o * tanh(c1)
#
# Strategy: tensor-parallel over the hidden (output-column) dim across 8
# cores (HS = 256 columns each); everything after the matmuls is local to a
# shard, so no collectives.
#
# Key design points (from perfetto/NTFF trace analysis; baseline bf16
# kernel = 43.7us, this kernel ~29us):
#  * Gate weights host-quantized to fp8 E3M4 (4 mantissa bits), x64
#    pre-scale (x128 for the g block so one tanh(x/(2*64)) serves
#    sigmoid(i,o) and tanh(g) alike).  E3M4 streams the PE at the full
#    bf16 rate (1 col/cycle) and halves HBM bytes vs bf16; E4M3 for the
#    gates fails the 2e-2 gate (measures 2.4e-2).  End-to-end err ~1.2e-2.
#  * The alpha matmuls (x@aW_ih, c@aW_hh) run in DoubleRow (double-pumped)
#    fp8 E4M3 at 2 k-chunks per pass — the alpha path is ~3x less error-
#    sensitive than the gates, so E4M3 is safe there (operand scales:
#    x*4 @ 16*aW_ih and 2*c @ 32*aW_hh keep products at x64).
#  * All sigmoids via 0.5+0.5*tanh(x/2): tanh and exp share one ACT table
#    set, so no ~2.7us mid-kernel table reloads.
#  * PE p-states: 0.65 -> 1.2 -> 2.4 GHz with ~3us of GAPLESS activity
#    needed to reach max; any stall during the ramp resets it (a ~300ns
#    just-in-time chunk wait cost an early version 11us by pinning the
#    whole stream at 1.2 GHz).  Once at 2.4 GHz, ordinary gaps are fine
#    (clock only drops after ~3.4us idle).  19 dummy matmuls on memset
#    data bridge engine-init (~7.5us) to the first chunk semaphore
#    (~11us) with zero gaps.
#  * DMA reality on this part: each dma_start costs ~0.65us of SP
#    sequencer time, completion semaphores trail the data by ~1.3us
#    (+~3us extra for the very first transfers, absorbed here by a 64KB
#    throwaway warm-up read), and the stream sustains ~330-375 GB/s only
#    for >=0.5-1MB transfers.  So: one warm-up read, then chunk sizes
#    that grow from 0.26MB (fast first semaphore) to ~1MB, issued in
#    exact PE consumption order on the sync ring only — merely touching
#    the scalar HWDGE ring adds ~2.2us of runtime queue setup.
#  * PE order ig -> alpha -> o: the [i;alpha] exp-normalize closes as a
#    single K=65 ones-matmul, c1 and tanh(c1) are computed while the PE
#    streams the o gate, and the post-PE tail is only
#    tanh(o)->affine->mul->store.  Bias rows join as rank-1 matmuls at
#    the accumulation-group edges (no extra serial ops).
#  * Fixed costs measured on this setup: ~7us framework preamble + ~3us
#    final-DMA+teardown; a 1-DMA kernel measures 13.5us end to end.

import numpy as np

import concourse.bass as bass
import concourse.tile as tile
from concourse import bacc, mybir
from concourse.bass_utils import run_bass_kernel_spmd

NCORES = 8
H = 2048          # hidden size
IN = 2048         # input size
C = 64            # number of skip-word cell states
HS = H // NCORES  # hidden shard per core = 256
KG = IN + H       # gates contraction dim = 4096
KO_G = KG // 128  # 32 k-chunks for gates
KO_A = IN // 128  # 16 k-chunks per alpha matmul
PAIRS = KO_A // 2  # 8 DoubleRow passes per alpha matmul
SCALE = 64.0      # fp8 weight pre-scale
F32 = mybir.dt.float32
F32R = mybir.dt.float32r
BF16 = mybir.dt.bfloat16
FP8 = mybir.dt.float8e3
FP8E4 = mybir.dt.float8e4

_nc_cache = None


def _build_nc():
    """Build the single-core Bass program (same program runs on all 8 cores)."""
    nc = bacc.Bacc(
        "TRN2",
        target_bir_lowering=False,
        debug=False,
        enable_asserts=False,
        name="multi_input_lstm_cell",
    )

    # DRAM I/O (per-core shards; shapes identical on every core).  Weights
    # host-pre-tiled to [ki=128, ko, n]: chunk DMAs read one contiguous
    # segment per partition.
    # wig columns: [i-gate shard (256) | 2*g-gate shard (256) | xt bytes],
    # e3m4 * 64; cols 512:514 carry the bf16 [h0|input_] value for chunk kk
    # so the gates lhsT arrives with the first weight chunk (one fewer DMA)
    wig = nc.dram_tensor("wig", [128, KO_G, 2 * HS + 2], FP8,
                         kind="ExternalInput").ap()
    wo = nc.dram_tensor("wo", [128, KO_G, HS], FP8, kind="ExternalInput").ap()
    # wa8: DoubleRow-interleaved alpha weights, pairs 0..7 = 16*aW_ih shard,
    # pairs 8..15 = 32*aW_hh shard (e4m3)
    wa8 = nc.dram_tensor("wa8", [128, 2 * PAIRS, 2, HS], FP8E4,
                         kind="ExternalInput").ap()
    # ctxa: cols 0:64 = x2 c_input^T pairs (alpha-hh lhsT), col 64 = x4
    # input_ pairs (wi lhsT), cols 65:80 pad -> the 80 B k-pair stride is
    # a multiple of 16 as DoubleRow LDWEIGHTS requires.
    ctxa = nc.dram_tensor("ctxa", [128, PAIRS, 2, 80], FP8E4,
                          kind="ExternalInput").ap()
    # bab = [64*b_i | 128*b_g | 64*b_o | 64*ab]
    bab = nc.dram_tensor("bab", [1, 4 * HS], F32R, kind="ExternalInput").ap()
    cs = nc.dram_tensor("cs", [C, HS], F32R, kind="ExternalInput").ap()
    # hc[0, 0:256] = c1 shard, hc[0, 256:512] = h1 shard
    hc = nc.dram_tensor("hc", [1, 2 * HS], F32, kind="ExternalOutput").ap()

    with tile.TileContext(nc) as tc:
        _emit(tc, wig, wo, wa8, ctxa, bab, cs, hc)

    nc.compile()
    return nc


def _emit(tc, wig, wo, wa8, ctxa, bab, cs, hc):
    from contextlib import ExitStack

    nc = tc.nc
    TANH = mybir.ActivationFunctionType.Tanh
    EXP = mybir.ActivationFunctionType.Exp
    MUL = mybir.AluOpType.mult
    ADD = mybir.AluOpType.add
    DR = mybir.MatmulPerfMode.DoubleRow
    INV2S = 1.0 / (2.0 * SCALE)

    with ExitStack() as ctx:
        singles = ctx.enter_context(tc.tile_pool(name="singles", bufs=1))
        psum = ctx.enter_context(tc.tile_pool(name="psum", bufs=1, space="PSUM"))

        # ---- memset-sourced tiles (no DMA dependency) --------------------
        warm_t = singles.tile([128, HS], BF16, tag="warm")
        nc.vector.memset(warm_t[:], 0.0)
        ones_b_f = singles.tile([1, C], F32, tag="ones_b")
        nc.vector.memset(ones_b_f[:], 1.0)
        ones_b = ones_b_f[:].bitcast(F32R)
        ones_r_f = singles.tile([C + 1, 1], F32, tag="ones_r")
        nc.vector.memset(ones_r_f[:], 1.0)
        ones_r = ones_r_f[:].bitcast(F32R)
        wmt = singles.tile([1, 1], F32, tag="wmt")
        nc.vector.memset(wmt[:], 0.0)
        nc.scalar.activation(out=wmt[:], in_=wmt[:], func=EXP)  # table prewarm
        half1 = singles.tile([1, 1], F32, tag="half1")
        nc.vector.memset(half1[:], 0.5)
        half64 = singles.tile([C, 1], F32, tag="half64")
        nc.vector.memset(half64[:], 0.5)

        # ---- weight/side tiles (whole tensors; DMAs write disjoint slices,
        # so no pool recycling and no backpressure on the issue stream) ----
        wig_t = singles.tile([128, KO_G, 2 * HS + 2], FP8, tag="wig")
        xt_t = wig_t[:, :, 2 * HS : 2 * HS + 2].bitcast(BF16)  # [128, 32, 1]
        wa8_t = singles.tile([128, 2 * PAIRS, 2, HS], FP8E4, tag="wa8")
        ctxa_t = singles.tile([128, PAIRS, 2, 80], FP8E4, tag="ctxa")
        cs_t = singles.tile([C, HS], F32R, tag="cs")
        bab_t = singles.tile([1, 4 * HS], F32R, tag="bab")
        wo_t = singles.tile([128, KO_G, HS], FP8, tag="wo")

        # sync-ring stream, in PE consumption order.  (The DMA path has
        # its own ~3us clock ramp after engine init and first-transfer
        # completion semaphores never fire before ~12us no matter how small
        # the transfer, so the schedule just rides it: small first chunks,
        # growing sizes, and the PE's dummy block sized to meet the first
        # semaphore.)
        for a, b in [(0, 2), (2, 6), (6, 14), (14, 23), (23, 32)]:
            nc.sync.dma_start(out=wig_t[:, a:b, :], in_=wig[:, a:b, :])
        nc.sync.dma_start(out=wa8_t[:, 0:PAIRS], in_=wa8[:, 0:PAIRS])
        nc.sync.dma_start(out=ctxa_t[:], in_=ctxa)
        nc.sync.dma_start(out=wa8_t[:, PAIRS : 2 * PAIRS],
                          in_=wa8[:, PAIRS : 2 * PAIRS])
        nc.sync.dma_start(out=cs_t[:], in_=cs)
        nc.sync.dma_start(out=bab_t[:], in_=bab)
        nc.sync.dma_start(out=wo_t[:, 0:16, :], in_=wo[:, 0:16, :])
        nc.sync.dma_start(out=wo_t[:, 16:26, :], in_=wo[:, 16:26, :])
        nc.sync.dma_start(out=wo_t[:, 26:32, :], in_=wo[:, 26:32, :])

        # ---- PSUM tiles --------------------------------------------------
        pg_ig = psum.tile([1, 2 * HS], F32, tag="pg_ig")  # [pre_i | 2*pre_g]
        pg_o = psum.tile([1, HS], F32, tag="pg_o")        # pre_o
        pwi = psum.tile([1, HS], F32, tag="pwi")          # alpha_wi row
        pal = psum.tile([C, HS], F32, tag="pal")          # alpha pre-activation
        ps = psum.tile([1, 2 * HS], F32, tag="ps")        # [sum_ew | sum_mg]
        pdum = psum.tile([1, HS], F32, tag="pdum")        # warm scratch

        # ---- PE ramp dummies (gap-free by construction; see header) ------
        for _ in range(25):
            nc.tensor.matmul(pdum[:], lhsT=warm_t[:, 0:1], rhs=warm_t[:],
                             start=True, stop=True)

        # ---- [i|g] gates stream (e3m4, single-rate) ----------------------
        for kk in range(KO_G):
            nc.tensor.matmul(
                pg_ig[:],
                lhsT=xt_t[:, kk, :],
                rhs=wig_t[:, kk, 0 : 2 * HS],
                start=(kk == 0),
                stop=False,
            )
        # bias joins as a K=1 rank-1 matmul closing the group (keeps the
        # first gates matmul free of the bias-DMA dependency)
        nc.tensor.matmul(pg_ig[:], lhsT=ones_b[0:1, 0:1], rhs=bab_t[:, 0:512],
                         start=False, stop=True)

        # ---- ig tail: row 64 of the normalize operands -------------------
        # tio = tanh(pre/(2S)) = [t_i | tanh(pre_g)]
        tio = singles.tile([1, 2 * HS], F32, tag="tio")
        nc.scalar.activation(out=tio[:], in_=pg_ig[:], func=TANH, scale=INV2S)
        ew_t = singles.tile([C + 1, HS], F32R, tag="ew")
        mg_t = singles.tile([C + 1, HS], F32R, tag="mg")
        # ew[64] = exp(sig_i) = exp(0.5*t_i + 0.5); mg[64] = g * ew[64]
        # (two steps: DVE tensor_tensor needs equal SBUF base partitions)
        nc.scalar.activation(out=ew_t[C : C + 1, :], in_=tio[:, 0:HS], func=EXP,
                             scale=0.5, bias=half1[:])
        nc.vector.tensor_scalar(out=mg_t[C : C + 1, :], in0=tio[:, HS : 2 * HS],
                                scalar1=1.0, scalar2=None, op0=MUL)
        nc.vector.tensor_tensor(out=mg_t[C : C + 1, :], in0=mg_t[C : C + 1, :],
                                in1=ew_t[C : C + 1, :], op=MUL)

        # ---- alpha matmuls: DoubleRow fp8, 2 k-chunks per pass -----------
        for p in range(PAIRS):
            nc.tensor.matmul(
                pwi[:],
                lhsT=ctxa_t[:, p, :, C : C + 1],
                rhs=wa8_t[:, p, :, :],
                start=(p == 0),
                stop=(p == PAIRS - 1),
                perf_mode=DR,
            )
        for p in range(PAIRS):
            nc.tensor.matmul(
                pal[:],
                lhsT=ctxa_t[:, p, :, 0:C],
                rhs=wa8_t[:, PAIRS + p, :, :],
                start=(p == 0),
                stop=False,
                perf_mode=DR,
            )
        # wi row (+ alpha_bias) to SBUF, broadcast-added into pal via a K=1
        # ones matmul (closes the pal group).
        wi_t = singles.tile([1, HS], F32R, tag="wi")
        nc.vector.tensor_tensor(out=wi_t[:], in0=pwi[:], in1=bab_t[:, 768:1024],
                                op=ADD)
        nc.tensor.matmul(pal[:], lhsT=ones_b[0:1, 0:C], rhs=wi_t[:],
                         start=False, stop=True)

        # ---- alpha tail (ACT/DVE; overlaps the o stream) -----------------
        tal = singles.tile([C, HS], F32, tag="tal")
        nc.scalar.activation(out=tal[:], in_=pal[:], func=TANH, scale=INV2S)
        nc.scalar.activation(out=ew_t[0:C, :], in_=tal[:], func=EXP,
                             scale=0.5, bias=half64[:])
        nc.vector.tensor_tensor(out=mg_t[0:C, :], in0=cs_t[:], in1=ew_t[0:C, :],
                                op=MUL)

        # ---- o-gate stream with the K=65 reductions interleaved ----------
        nc.tensor.matmul(pg_o[:], lhsT=ones_b[0:1, 0:1], rhs=bab_t[:, 512:768],
                         start=True, stop=False)
        for kk in range(KO_G):
            nc.tensor.matmul(
                pg_o[:],
                lhsT=xt_t[:, kk, :],
                rhs=wo_t[:, kk, :],
                start=False,
                stop=(kk == KO_G - 1),
            )
            if kk == 10:
                nc.tensor.matmul(ps[:, 0:HS], lhsT=ones_r[:], rhs=ew_t[:],
                                 start=True, stop=True)
                nc.tensor.matmul(ps[:, HS : 2 * HS], lhsT=ones_r[:], rhs=mg_t[:],
                                 start=True, stop=True)

        # ---- c1 = ps1 / ps0 ; overlaps the o-stream tail -----------------
        r_t = singles.tile([1, HS], F32, tag="r")
        nc.vector.reciprocal_approx_fast(out=r_t[:], in_=ps[:, 0:HS])
        hc_t = singles.tile([1, 2 * HS], F32, tag="hc")
        nc.vector.tensor_tensor(out=hc_t[:, 0:HS], in0=ps[:, HS : 2 * HS],
                                in1=r_t[:], op=MUL)
        nc.sync.dma_start(out=hc[:, 0:HS], in_=hc_t[:, 0:HS])
        t4 = singles.tile([1, HS], F32, tag="t4")
        nc.scalar.activation(out=t4[:], in_=hc_t[:, 0:HS], func=TANH)

        # ---- o tail: h1 = (0.5 + 0.5*tanh(pre_o/(2S))) * tanh(c1) --------
        to_t = singles.tile([1, HS], F32, tag="to")
        nc.scalar.activation(out=to_t[:], in_=pg_o[:], func=TANH, scale=INV2S)
        nc.vector.tensor_scalar(out=to_t[:], in0=to_t[:], scalar1=0.5,
                                scalar2=0.5, op0=MUL, op1=ADD)
        nc.vector.tensor_tensor(out=hc_t[:, HS : 2 * HS], in0=to_t[:],
                                in1=t4[:], op=MUL)
        nc.sync.dma_start(out=hc[:, HS : 2 * HS], in_=hc_t[:, HS : 2 * HS])


def _shard_inputs(input_, c_input, h0, c0, weight_ih, weight_hh,
                  alpha_weight_ih, alpha_weight_hh, bias, alpha_bias):
    """Host-side scatter: column-shard the weights over the hidden dim."""
    import ml_dtypes
    f32 = np.float32
    bf16 = ml_dtypes.bfloat16
    e3m4 = ml_dtypes.float8_e3m4
    e4m3 = ml_dtypes.float8_e4m3

    x_comb = np.concatenate([h0[0], input_[0]]).astype(f32)          # (4096,)
    xt = np.ascontiguousarray(x_comb.reshape(KO_G, 128).T).astype(bf16)
    xt_bytes = xt.view(np.uint8).reshape(128, KO_G, 2).view(e3m4)
    # ctxa: cols 0:64 = x2 c_input^T pairs, col 64 = x4 input_ pairs
    ctxa = np.zeros((128, PAIRS, 2, 80), e3m4)  # raw bytes; dtype irrelevant
    ctxa[:, :, :, 0:C] = np.ascontiguousarray(
        (c_input.T.astype(f32) * f32(2.0)).reshape(PAIRS, 2, 128, C)
        .transpose(2, 0, 1, 3)).astype(e4m3).view(e3m4)
    ctxa[:, :, :, C] = np.ascontiguousarray(
        (input_[0].astype(f32) * f32(4.0)).reshape(PAIRS, 2, 128)
        .transpose(2, 0, 1)).astype(e4m3).view(e3m4)
    ctxa = ctxa.view(e4m3)

    def q8(x, sc):
        return np.clip(np.asarray(x, f32) * f32(sc), -15.5, 15.5).astype(e3m4)

    # gates weights: stack [W_hh; W_ih]; i/o at x64, g at x128 (e3m4).
    wg_full = np.concatenate([weight_hh, weight_ih], axis=0).astype(f32)
    wq_i = q8(wg_full[:, 0:H], SCALE)
    wq_o = q8(wg_full[:, H : 2 * H], SCALE)
    wq_g = q8(wg_full[:, 2 * H : 3 * H], 2 * SCALE)
    del wg_full

    def tile_k(w):  # [4096, n] -> [128, 32, n]
        n = w.shape[1]
        return np.ascontiguousarray(w.reshape(KO_G, 128, n).transpose(1, 0, 2))

    # alpha weights for DoubleRow (e4m3): ih at x16 (against x4 inputs),
    # hh at x32 (against x2 c_input) -> products uniformly x64.
    wa_q = np.concatenate(
        [(alpha_weight_ih.astype(f32) * f32(SCALE / 4)).astype(e4m3),
         (alpha_weight_hh.astype(f32) * f32(SCALE / 2)).astype(e4m3)], axis=0)

    bias = np.asarray(bias, f32)
    alpha_bias = np.asarray(alpha_bias, f32)
    c_input = np.asarray(c_input, f32)

    in_maps = []
    for k in range(NCORES):
        cols = np.s_[k * HS : (k + 1) * HS]
        wig = np.zeros((128, KO_G, 2 * HS + 2), e3m4)
        wig[:, :, 0 : 2 * HS] = tile_k(
            np.concatenate([wq_i[:, cols], wq_g[:, cols]], axis=1))
        wig[:, :, 2 * HS : 2 * HS + 2] = xt_bytes
        wo = tile_k(wq_o[:, cols])
        # [4096, 256] -> [128, 16 pairs, 2, 256]
        wa8 = np.ascontiguousarray(
            wa_q[:, cols].reshape(2 * PAIRS, 2, 128, HS).transpose(2, 0, 1, 3))
        bab = np.concatenate(
            [bias[0 * H + k * HS : 0 * H + (k + 1) * HS] * f32(SCALE),
             bias[2 * H + k * HS : 2 * H + (k + 1) * HS] * f32(2 * SCALE),
             bias[1 * H + k * HS : 1 * H + (k + 1) * HS] * f32(SCALE),
             alpha_bias[cols] * f32(SCALE)])[None, :].astype(f32)
        in_maps.append({
            "wig": wig,
            "wo": wo,
            "wa8": wa8,
            "ctxa": ctxa,
            "bab": bab,
            "cs": np.ascontiguousarray(c_input[:, cols]),
        })
    return in_maps


def _run(inputs, trace=False):
    global _nc_cache
    if _nc_cache is None:
        _nc_cache = _build_nc()
    nc = _nc_cache
    in_maps = _shard_inputs(**inputs)
    res = run_bass_kernel_spmd(nc, in_maps, core_ids=list(range(NCORES)), trace=trace)
    h1 = np.concatenate(
        [res.results[k]["hc"][:, HS : 2 * HS] for k in range(NCORES)], axis=1)
    c1 = np.concatenate(
        [res.results[k]["hc"][:, 0:HS] for k in range(NCORES)], axis=1)
    return (h1.astype(np.float32), c1.astype(np.float32)), res


def kernel(input_, c_input, h0, c0, weight_ih, weight_hh,
           alpha_weight_ih, alpha_weight_hh, bias, alpha_bias):
    inputs = dict(
        input_=np.asarray(input_, np.float32),
        c_input=np.asarray(c_input, np.float32),
        h0=np.asarray(h0, np.float32),
        c0=np.asarray(c0, np.float32),
        weight_ih=np.asarray(weight_ih, np.float32),
        weight_hh=np.asarray(weight_hh, np.float32),
        alpha_weight_ih=np.asarray(alpha_weight_ih, np.float32),
        alpha_weight_hh=np.asarray(alpha_weight_hh, np.float32),
        bias=np.asarray(bias, np.float32),
        alpha_bias=np.asarray(alpha_bias, np.float32),
    )
    out, _ = _run(inputs)
    return out


# revision 27
# speedup vs baseline: 1.0350x; 1.0002x over previous
# Self-contained Trainium2 Bass kernel for nn_MultiInputLSTMCell.
#
# Reference computation (all fp32):
#   pre   = h0 @ W_hh + bias + input_ @ W_ih          # (1, 3H)
#   i, o  = sigmoid(pre[:, :H]), sigmoid(pre[:, H:2H])
#   g     = tanh(pre[:, 2H:])
#   awi   = input_ @ aW_ih + a_bias                   # (1, H)
#   awh   = c_input @ aW_hh                           # (C, H)
#   alpha = sigmoid(awi + awh)                        # (C, H)
#   w     = exp([i; alpha]); w /= w.sum(0)            # (C+1, H)
#   c1    = (([g; c_input]) * w).sum(0)               # (1, H)
#   h1    = o * tanh(c1)
#
# Strategy: tensor-parallel over the hidden (output-column) dim across 8
# cores (HS = 256 columns each); everything after the matmuls is local to a
# shard, so no collectives.
#
# Key design points (from perfetto/NTFF trace analysis; baseline bf16
# kernel = 43.7us, this kernel ~29us):
#  * Gate weights host-quantized to fp8 E3M4 (4 mantissa bits), x64
#    pre-scale (x128 for the g block so one tanh(x/(2*64)) serves
#    sigmoid(i,o) and tanh(g) alike).  E3M4 streams the PE at the full
#    bf16 rate (1 col/cycle) and halves HBM bytes vs bf16; E4M3 for the
#    gates fails the 2e-2 gate (measures 2.4e-2).  End-to-end err ~1.2e-2.
#  * The alpha matmuls (x@aW_ih, c@aW_hh) run in DoubleRow (double-pumped)
#    fp8 E4M3 at 2 k-chunks per pass — the alpha path is ~3x less error-
#    sensitive than the gates, so E4M3 is safe there (operand scales:
#    x*4 @ 16*aW_ih and 2*c @ 32*aW_hh keep products at x64).
#  * All sigmoids via 0.5+0.5*tanh(x/2): tanh and exp share one ACT table
#    set, so no ~2.7us mid-kernel table reloads.
#  * PE p-states: 0.65 -> 1.2 -> 2.4 GHz with ~3us of GAPLESS activity
#    needed to reach max; any stall during the ramp resets it (a ~300ns
#    just-in-time chunk wait cost an early version 11us by pinning the
#    whole stream at 1.2 GHz).  Once at 2.4 GHz, ordinary gaps are fine
#    (clock only drops after ~3.4us idle).  19 dummy matmuls on memset
#    data bridge engine-init (~7.5us) to the first chunk semaphore
#    (~11us) with zero gaps.
#  * DMA reality on this part: each dma_start costs ~0.65us of SP
#    sequencer time, completion semaphores trail the data by ~1.3us
#    (+~3us extra for the very first transfers, absorbed here by a 64KB
#    throwaway warm-up read), and the stream sustains ~330-375 GB/s only
#    for >=0.5-1MB transfers.  So: one warm-up read, then chunk sizes
#    that grow from 0.26MB (fast first semaphore) to ~1MB, issued in
#    exact PE consumption order on the sync ring only — merely touching
#    the scalar HWDGE ring adds ~2.2us of runtime queue setup.
#  * PE order ig -> alpha -> o: the [i;alpha] exp-normalize closes as a
#    single K=65 ones-matmul, c1 and tanh(c1) are computed while the PE
#    streams the o gate, and the post-PE tail is only
#    tanh(o)->affine->mul->store.  Bias rows join as rank-1 matmuls at
#    the accumulation-group edges (no extra serial ops).
#  * Fixed costs measured on this setup: ~7us framework preamble + ~3us
#    final-DMA+teardown; a 1-DMA kernel measures 13.5us end to end.

import numpy as np

import concourse.bass as bass
import concourse.tile as tile
from concourse import bacc, mybir
from concourse.bass_utils import run_bass_kernel_spmd

NCORES = 8
H = 2048          # hidden size
IN = 2048         # input size
C = 64            # number of skip-word cell states
HS = H // NCORES  # hidden shard per core = 256
KG = IN + H       # gates contraction dim = 4096
KO_G = KG // 128  # 32 k-chunks for gates
KO_A = IN // 128  # 16 k-chunks per alpha matmul
PAIRS = KO_A // 2  # 8 DoubleRow passes per alpha matmul
SCALE = 64.0      # fp8 weight pre-scale
F32 = mybir.dt.float32
F32R = mybir.dt.float32r
BF16 = mybir.dt.bfloat16
FP8 = mybir.dt.float8e3
FP8E4 = mybir.dt.float8e4

_nc_cache = None


def _build_nc():
    """Build the single-core Bass program (same program runs on all 8 cores)."""
    nc = bacc.Bacc(
        "TRN2",
        target_bir_lowering=False,
        debug=False,
        enable_asserts=False,
        name="multi_input_lstm_cell",
    )

    # DRAM I/O (per-core shards; shapes identical on every core).  Weights
    # host-pre-tiled to [ki=128, ko, n]: chunk DMAs read one contiguous
    # segment per partition.
    # wig columns: [i-gate shard (256) | 2*g-gate shard (256) | xt bytes],
    # e3m4 * 64; cols 512:514 carry the bf16 [h0|input_] value for chunk kk
    # so the gates lhsT arrives with the first weight chunk (one fewer DMA)
    wig = nc.dram_tensor("wig", [128, KO_G, 2 * HS + 2], FP8,
                         kind="ExternalInput").ap()
    wo = nc.dram_tensor("wo", [128, KO_G, HS], FP8, kind="ExternalInput").ap()
    # wa8: DoubleRow-interleaved alpha weights, pairs 0..7 = 16*aW_ih shard,
    # pairs 8..15 = 32*aW_hh shard (e4m3)
    wa8 = nc.dram_tensor("wa8", [128, 2 * PAIRS, 2, HS], FP8E4,
                         kind="ExternalInput").ap()
    # ctxa: cols 0:64 = x2 c_input^T pairs (alpha-hh lhsT), col 64 = x4
    # input_ pairs (wi lhsT), cols 65:80 pad -> the 80 B k-pair stride is
    # a multiple of 16 as DoubleRow LDWEIGHTS requires.
    ctxa = nc.dram_tensor("ctxa", [128, PAIRS, 2, 80], FP8E4,
                          kind="ExternalInput").ap()
    # bab = [64*b_i | 128*b_g | 64*b_o | 64*ab]
    bab = nc.dram_tensor("bab", [1, 4 * HS], F32R, kind="ExternalInput").ap()
    cs = nc.dram_tensor("cs", [C, HS], F32R, kind="ExternalInput").ap()
    # hc[0, 0:256] = c1 shard, hc[0, 256:512] = h1 shard
    hc = nc.dram_tensor("hc", [1, 2 * HS], F32, kind="ExternalOutput").ap()

    with tile.TileContext(nc) as tc:
        _emit(tc, wig, wo, wa8, ctxa, bab, cs, hc)

    nc.compile()
    return nc


def _emit(tc, wig, wo, wa8, ctxa, bab, cs, hc):
    from contextlib import ExitStack

    nc = tc.nc
    TANH = mybir.ActivationFunctionType.Tanh
    EXP = mybir.ActivationFunctionType.Exp
    MUL = mybir.AluOpType.mult
    ADD = mybir.AluOpType.add
    DR = mybir.MatmulPerfMode.DoubleRow
    INV2S = 1.0 / (2.0 * SCALE)

    with ExitStack() as ctx:
        singles = ctx.enter_context(tc.tile_pool(name="singles", bufs=1))
        psum = ctx.enter_context(tc.tile_pool(name="psum", bufs=1, space="PSUM"))

        # ---- memset-sourced tiles (no DMA dependency) --------------------
        warm_t = singles.tile([128, HS], BF16, tag="warm")
        nc.vector.memset(warm_t[:], 0.0)
        ones_b_f = singles.tile([1, C], F32, tag="ones_b")
        nc.vector.memset(ones_b_f[:], 1.0)
        ones_b = ones_b_f[:].bitcast(F32R)
        ones_r_f = singles.tile([C + 1, 1], F32, tag="ones_r")
        nc.vector.memset(ones_r_f[:], 1.0)
        ones_r = ones_r_f[:].bitcast(F32R)
        wmt = singles.tile([1, 1], F32, tag="wmt")
        nc.vector.memset(wmt[:], 0.0)
        nc.scalar.activation(out=wmt[:], in_=wmt[:], func=EXP)  # table prewarm
        half1 = singles.tile([1, 1], F32, tag="half1")
        nc.vector.memset(half1[:], 0.5)
        half64 = singles.tile([C, 1], F32, tag="half64")
        nc.vector.memset(half64[:], 0.5)

        # ---- weight/side tiles (whole tensors; DMAs write disjoint slices,
        # so no pool recycling and no backpressure on the issue stream) ----
        wig_t = singles.tile([128, KO_G, 2 * HS + 2], FP8, tag="wig")
        xt_t = wig_t[:, :, 2 * HS : 2 * HS + 2].bitcast(BF16)  # [128, 32, 1]
        wa8_t = singles.tile([128, 2 * PAIRS, 2, HS], FP8E4, tag="wa8")
        ctxa_t = singles.tile([128, PAIRS, 2, 80], FP8E4, tag="ctxa")
        cs_t = singles.tile([C, HS], F32R, tag="cs")
        bab_t = singles.tile([1, 4 * HS], F32R, tag="bab")
        wo_t = singles.tile([128, KO_G, HS], FP8, tag="wo")

        # sync-ring stream, in PE consumption order.  (The DMA path has
        # its own ~3us clock ramp after engine init and first-transfer
        # completion semaphores never fire before ~12us no matter how small
        # the transfer, so the schedule just rides it: small first chunks,
        # growing sizes, and the PE's dummy block sized to meet the first
        # semaphore.)
        for a, b in [(0, 2), (2, 6), (6, 14), (14, 22), (22, 32)]:
            nc.sync.dma_start(out=wig_t[:, a:b, :], in_=wig[:, a:b, :])
        nc.sync.dma_start(out=wa8_t[:, 0:PAIRS], in_=wa8[:, 0:PAIRS])
        nc.sync.dma_start(out=ctxa_t[:], in_=ctxa)
        nc.sync.dma_start(out=wa8_t[:, PAIRS : 2 * PAIRS],
                          in_=wa8[:, PAIRS : 2 * PAIRS])
        nc.sync.dma_start(out=cs_t[:], in_=cs)
        nc.sync.dma_start(out=bab_t[:], in_=bab)
        nc.sync.dma_start(out=wo_t[:, 0:14, :], in_=wo[:, 0:14, :])
        nc.sync.dma_start(out=wo_t[:, 14:24, :], in_=wo[:, 14:24, :])
        nc.sync.dma_start(out=wo_t[:, 24:32, :], in_=wo[:, 24:32, :])

        # ---- PSUM tiles --------------------------------------------------
        pg_ig = psum.tile([1, 2 * HS], F32, tag="pg_ig")  # [pre_i | 2*pre_g]
        pg_o = psum.tile([1, HS], F32, tag="pg_o")        # pre_o
        pwi = psum.tile([1, HS], F32, tag="pwi")          # alpha_wi row
        pal = psum.tile([C, HS], F32, tag="pal")          # alpha pre-activation
        ps = psum.tile([1, 2 * HS], F32, tag="ps")        # [sum_ew | sum_mg]
        pdum = psum.tile([1, HS], F32, tag="pdum")        # warm scratch

        # ---- PE ramp dummies (gap-free by construction; see header) ------
        for _ in range(25):
            nc.tensor.matmul(pdum[:], lhsT=warm_t[:, 0:1], rhs=warm_t[:],
                             start=True, stop=True)

        # ---- [i|g] gates stream (e3m4, single-rate) ----------------------
        for kk in range(KO_G):
            nc.tensor.matmul(
                pg_ig[:],
                lhsT=xt_t[:, kk, :],
                rhs=wig_t[:, kk, 0 : 2 * HS],
                start=(kk == 0),
                stop=False,
            )
        # bias joins as a K=1 rank-1 matmul closing the group (keeps the
        # first gates matmul free of the bias-DMA dependency)
        nc.tensor.matmul(pg_ig[:], lhsT=ones_b[0:1, 0:1], rhs=bab_t[:, 0:512],
                         start=False, stop=True)

        # ---- ig tail: row 64 of the normalize operands -------------------
        # tio = tanh(pre/(2S)) = [t_i | tanh(pre_g)]
        tio = singles.tile([1, 2 * HS], F32, tag="tio")
        nc.scalar.activation(out=tio[:], in_=pg_ig[:], func=TANH, scale=INV2S)
        ew_t = singles.tile([C + 1, HS], F32R, tag="ew")
        mg_t = singles.tile([C + 1, HS], F32R, tag="mg")
        # ew[64] = exp(sig_i) = exp(0.5*t_i + 0.5); mg[64] = g * ew[64]
        # (two steps: DVE tensor_tensor needs equal SBUF base partitions)
        nc.scalar.activation(out=ew_t[C : C + 1, :], in_=tio[:, 0:HS], func=EXP,
                             scale=0.5, bias=half1[:])
        nc.vector.tensor_scalar(out=mg_t[C : C + 1, :], in0=tio[:, HS : 2 * HS],
                                scalar1=1.0, scalar2=None, op0=MUL)
        nc.vector.tensor_tensor(out=mg_t[C : C + 1, :], in0=mg_t[C : C + 1, :],
                                in1=ew_t[C : C + 1, :], op=MUL)

        # ---- alpha matmuls: DoubleRow fp8, 2 k-chunks per pass -----------
        for p in range(PAIRS):
            nc.tensor.matmul(
                pwi[:],
                lhsT=ctxa_t[:, p, :, C : C + 1],
                rhs=wa8_t[:, p, :, :],
                start=(p == 0),
                stop=(p == PAIRS - 1),
                perf_mode=DR,
            )
        for p in range(PAIRS):
            nc.tensor.matmul(
                pal[:],
                lhsT=ctxa_t[:, p, :, 0:C],
                rhs=wa8_t[:, PAIRS + p, :, :],
                start=(p == 0),
                stop=False,
                perf_mode=DR,
            )
        # wi row (+ alpha_bias) to SBUF, broadcast-added into pal via a K=1
        # ones matmul (closes the pal group).
        wi_t = singles.tile([1, HS], F32R, tag="wi")
        nc.vector.tensor_tensor(out=wi_t[:], in0=pwi[:], in1=bab_t[:, 768:1024],
                                op=ADD)
        nc.tensor.matmul(pal[:], lhsT=ones_b[0:1, 0:C], rhs=wi_t[:],
                         start=False, stop=True)

        # ---- alpha tail (ACT/DVE; overlaps the o stream) -----------------
        tal = singles.tile([C, HS], F32, tag="tal")
        nc.scalar.activation(out=tal[:], in_=pal[:], func=TANH, scale=INV2S)
        nc.scalar.activation(out=ew_t[0:C, :], in_=tal[:], func=EXP,
                             scale=0.5, bias=half64[:])
        nc.vector.tensor_tensor(out=mg_t[0:C, :], in0=cs_t[:], in1=ew_t[0:C, :],
                                op=MUL)

        # ---- o-gate stream with the K=65 reductions interleaved ----------
        nc.tensor.matmul(pg_o[:], lhsT=ones_b[0:1, 0:1], rhs=bab_t[:, 512:768],
                         start=True, stop=False)
        for kk in range(KO_G):
            nc.tensor.matmul(
                pg_o[:],
                lhsT=xt_t[:, kk, :],
                rhs=wo_t[:, kk, :],
                start=False,
                stop=(kk == KO_G - 1),
            )
            if kk == 10:
                nc.tensor.matmul(ps[:, 0:HS], lhsT=ones_r[:], rhs=ew_t[:],
                                 start=True, stop=True)
                nc.tensor.matmul(ps[:, HS : 2 * HS], lhsT=ones_r[:], rhs=mg_t[:],
                                 start=True, stop=True)

        # ---- c1 = ps1 / ps0 ; overlaps the o-stream tail -----------------
        r_t = singles.tile([1, HS], F32, tag="r")
        nc.vector.reciprocal_approx_fast(out=r_t[:], in_=ps[:, 0:HS])
        hc_t = singles.tile([1, 2 * HS], F32, tag="hc")
        nc.vector.tensor_tensor(out=hc_t[:, 0:HS], in0=ps[:, HS : 2 * HS],
                                in1=r_t[:], op=MUL)
        nc.sync.dma_start(out=hc[:, 0:HS], in_=hc_t[:, 0:HS])
        t4 = singles.tile([1, HS], F32, tag="t4")
        nc.scalar.activation(out=t4[:], in_=hc_t[:, 0:HS], func=TANH)

        # ---- o tail: h1 = (0.5 + 0.5*tanh(pre_o/(2S))) * tanh(c1) --------
        to_t = singles.tile([1, HS], F32, tag="to")
        nc.scalar.activation(out=to_t[:], in_=pg_o[:], func=TANH, scale=INV2S)
        nc.vector.tensor_scalar(out=to_t[:], in0=to_t[:], scalar1=0.5,
                                scalar2=0.5, op0=MUL, op1=ADD)
        nc.vector.tensor_tensor(out=hc_t[:, HS : 2 * HS], in0=to_t[:],
                                in1=t4[:], op=MUL)
        nc.sync.dma_start(out=hc[:, HS : 2 * HS], in_=hc_t[:, HS : 2 * HS])


def _shard_inputs(input_, c_input, h0, c0, weight_ih, weight_hh,
                  alpha_weight_ih, alpha_weight_hh, bias, alpha_bias):
    """Host-side scatter: column-shard the weights over the hidden dim."""
    import ml_dtypes
    f32 = np.float32
    bf16 = ml_dtypes.bfloat16
    e3m4 = ml_dtypes.float8_e3m4
    e4m3 = ml_dtypes.float8_e4m3

    x_comb = np.concatenate([h0[0], input_[0]]).astype(f32)          # (4096,)
    xt = np.ascontiguousarray(x_comb.reshape(KO_G, 128).T).astype(bf16)
    xt_bytes = xt.view(np.uint8).reshape(128, KO_G, 2).view(e3m4)
    # ctxa: cols 0:64 = x2 c_input^T pairs, col 64 = x4 input_ pairs
    ctxa = np.zeros((128, PAIRS, 2, 80), e3m4)  # raw bytes; dtype irrelevant
    ctxa[:, :, :, 0:C] = np.ascontiguousarray(
        (c_input.T.astype(f32) * f32(2.0)).reshape(PAIRS, 2, 128, C)
        .transpose(2, 0, 1, 3)).astype(e4m3).view(e3m4)
    ctxa[:, :, :, C] = np.ascontiguousarray(
        (input_[0].astype(f32) * f32(4.0)).reshape(PAIRS, 2, 128)
        .transpose(2, 0, 1)).astype(e4m3).view(e3m4)
    ctxa = ctxa.view(e4m3)

    def q8(x, sc):
        return np.clip(np.asarray(x, f32) * f32(sc), -15.5, 15.5).astype(e3m4)

    # gates weights: stack [W_hh; W_ih]; i/o at x64, g at x128 (e3m4).
    wg_full = np.concatenate([weight_hh, weight_ih], axis=0).astype(f32)
    wq_i = q8(wg_full[:, 0:H], SCALE)
    wq_o = q8(wg_full[:, H : 2 * H], SCALE)
    wq_g = q8(wg_full[:, 2 * H : 3 * H], 2 * SCALE)
    del wg_full

    def tile_k(w):  # [4096, n] -> [128, 32, n]
        n = w.shape[1]
        return np.ascontiguousarray(w.reshape(KO_G, 128, n).transpose(1, 0, 2))

    # alpha weights for DoubleRow (e4m3): ih at x16 (against x4 inputs),
    # hh at x32 (against x2 c_input) -> products uniformly x64.
    wa_q = np.concatenate(
        [(alpha_weight_ih.astype(f32) * f32(SCALE / 4)).astype(e4m3),
         (alpha_weight_hh.astype(f32) * f32(SCALE / 2)).astype(e4m3)], axis=0)

    bias = np.asarray(bias, f32)
    alpha_bias = np.asarray(alpha_bias, f32)
    c_input = np.asarray(c_input, f32)

    in_maps = []
    for k in range(NCORES):
        cols = np.s_[k * HS : (k + 1) * HS]
        wig = np.zeros((128, KO_G, 2 * HS + 2), e3m4)
        wig[:, :, 0 : 2 * HS] = tile_k(
            np.concatenate([wq_i[:, cols], wq_g[:, cols]], axis=1))
        wig[:, :, 2 * HS : 2 * HS + 2] = xt_bytes
        wo = tile_k(wq_o[:, cols])
        # [4096, 256] -> [128, 16 pairs, 2, 256]
        wa8 = np.ascontiguousarray(
            wa_q[:, cols].reshape(2 * PAIRS, 2, 128, HS).transpose(2, 0, 1, 3))
        bab = np.concatenate(
            [bias[0 * H + k * HS : 0 * H + (k + 1) * HS] * f32(SCALE),
             bias[2 * H + k * HS : 2 * H + (k + 1) * HS] * f32(2 * SCALE),
             bias[1 * H + k * HS : 1 * H + (k + 1) * HS] * f32(SCALE),
             alpha_bias[cols] * f32(SCALE)])[None, :].astype(f32)
        in_maps.append({
            "wig": wig,
            "wo": wo,
            "wa8": wa8,
            "ctxa": ctxa,
            "bab": bab,
            "cs": np.ascontiguousarray(c_input[:, cols]),
        })
    return in_maps


def _run(inputs, trace=False):
    global _nc_cache
    if _nc_cache is None:
        _nc_cache = _build_nc()
    nc = _nc_cache
    in_maps = _shard_inputs(**inputs)
    res = run_bass_kernel_spmd(nc, in_maps, core_ids=list(range(NCORES)), trace=trace)
    h1 = np.concatenate(
        [res.results[k]["hc"][:, HS : 2 * HS] for k in range(NCORES)], axis=1)
    c1 = np.concatenate(
        [res.results[k]["hc"][:, 0:HS] for k in range(NCORES)], axis=1)
    return (h1.astype(np.float32), c1.astype(np.float32)), res


def kernel(input_, c_input, h0, c0, weight_ih, weight_hh,
           alpha_weight_ih, alpha_weight_hh, bias, alpha_bias):
    inputs = dict(
        input_=np.asarray(input_, np.float32),
        c_input=np.asarray(c_input, np.float32),
        h0=np.asarray(h0, np.float32),
        c0=np.asarray(c0, np.float32),
        weight_ih=np.asarray(weight_ih, np.float32),
        weight_hh=np.asarray(weight_hh, np.float32),
        alpha_weight_ih=np.asarray(alpha_weight_ih, np.float32),
        alpha_weight_hh=np.asarray(alpha_weight_hh, np.float32),
        bias=np.asarray(bias, np.float32),
        alpha_bias=np.asarray(alpha_bias, np.float32),
    )
    out, _ = _run(inputs)
    return out
